# revision 14
# baseline (speedup 1.0000x reference)
"""Trainium2 Bass kernel for nn_Attentionv2 (B=8, N=1024, C=768, H=12, D=64).

Strategy: data-parallel over batch — one batch element per NeuronCore (8 cores).
Per core, multi-head attention is computed entirely in the "transposed"
orientation so no on-chip transposes are needed:

  QT[h*64+d, n] = sum_c WqT[c, h*64+d] * xT[c, n]     (head-pair tiles)
  KT likewise; V[n, h*64+d] = sum_c xT[c, n-tile] * WvT[c, :]
  ST[m, n]  = sum_d KT[d, m] * QT[d, n]               (scores transposed;
               the two heads of a pair sit on partitions 0-63 / 64-127 so
               their K=64 matmuls dual-stream on the two PE row groups)
  ET        = exp(ST * 1/8)                            (no max-subtraction:
                                                        scores are O(1) here)
  PV lhsT   = [V_h | ones(64 cols)]  =>  out rows 0-63 = OT_h (unnorm),
               rows 64-127 = softmax denominator replicated 64x
  OT_norm   = OT * recip(Z)
  y[n, o]   = sum_c OT_norm[c, n] * WpT[c, o] + bp[o]

Pipelining: the kernel is paced by the ACT engine (96 exp calls ~110us).
Every score step (one (j,mt) pair: 2 dual-stream MMs + 2 exps) is followed
by exactly one filler group on the PE — V-projection t-tiles during pair 0,
then PV(j-1) quarter-groups and QK(j+1) quarter-groups — so neither PE nor
ACT ever starves and HAM stays at full clock.  Input DMAs are split into
prioritized chunks (x halves + w chunk-groups on 4 queues) so the first
exp issues ~11us in instead of ~56us.  One PSUM pool with 3 tags
(s:4 banks, qk:2, o:2) covers all phases — the output projection reuses
the score banks, avoiding a pool-boundary drain.

Matmul operands are fp16 (full-rate PE); accumulation is fp32 in PSUM.
"""

import numpy as np

P = 128
B, N, C = 8, 1024, 768
H, D = 12, 64
SCALE = D ** -0.5  # 0.125
CT = C // P   # 6 contraction chunks
NT = N // P   # 8 sequence tiles
HP = H // 2   # 6 head pairs
NCORES = 8

_cache = {}


def _build_nc():
    import concourse.bass as bass
    import concourse.mybir as mybir
    import concourse.tile as tile
    from concourse import bacc

    f32 = mybir.dt.float32
    f16 = mybir.dt.float16
    Exp = mybir.ActivationFunctionType.Exp

    nc = bacc.Bacc("TRN2", target_bir_lowering=False, debug=False,
                   enable_asserts=False)

    xT = nc.dram_tensor("xT", [C, N], f16, kind="ExternalInput").ap()
    wqT = nc.dram_tensor("wqT", [C, H * D], f16, kind="ExternalInput").ap()
    wkT = nc.dram_tensor("wkT", [C, H * D], f16, kind="ExternalInput").ap()
    wvT = nc.dram_tensor("wvT", [C, H * D], f16, kind="ExternalInput").ap()
    wpT = nc.dram_tensor("wpT", [C, C], f16, kind="ExternalInput").ap()
    bpb = nc.dram_tensor("bpb", [P, C], f32, kind="ExternalInput").ap()
    y = nc.dram_tensor("y", [N, C], f32, kind="ExternalOutput").ap()

    mm = nc.tensor.matmul

    xTr = xT.rearrange("(o p) n -> p o n", p=P)
    wqTr = wqT.rearrange("(o p) f -> p o f", p=P)
    wkTr = wkT.rearrange("(o p) f -> p o f", p=P)
    wvTr = wvT.rearrange("(o p) f -> p o f", p=P)
    wpTr = wpT.rearrange("(o p) f -> p o f", p=P)

    with tile.TileContext(nc) as tc:
        with tc.tile_pool(name="persist", bufs=1) as persist, \
             tc.tile_pool(name="ph1", bufs=1) as ph1, \
             tc.tile_pool(name="ps", bufs=1, space="PSUM") as psp, \
             tc.tile_pool(name="et", bufs=24) as etp, \
             tc.tile_pool(name="sm", bufs=4) as smp, \
             tc.tile_pool(name="outp", bufs=3) as outp:
            qt = persist.tile([P, HP, N], f16)
            kt = persist.tile([P, HP, N], f16)
            vp = persist.tile([P, NT, H, 2 * D], f16)  # [Vh | ones]
            ot = persist.tile([P, HP, N], f16)
            wp_sb = persist.tile([P, CT, C], f16)
            bpb_sb = persist.tile([P, C], f32)

            x_sb = ph1.tile([P, CT, N], f16)
            wq_sb = ph1.tile([P, CT, H * D], f16)
            wk_sb = ph1.tile([P, CT, H * D], f16)
            wv_sb = ph1.tile([P, CT, H * D], f16)

            # --- prioritized chunked input DMAs on 4 queues.  x halves +
            # wq/wk chunk-groups first (QK(0) gates the first exp), wv next
            # (needed by the V-proj fillers from ~13us), wp/bpb last. ---
            nc.sync.dma_start(x_sb[:, 0:3, :], xTr[:, 0:3, :])
            nc.scalar.dma_start(wq_sb[:, 0:3, :], wqTr[:, 0:3, :])
            nc.gpsimd.dma_start(wk_sb[:, 0:3, :], wkTr[:, 0:3, :])
            nc.sync.dma_start(x_sb[:, 3:6, :], xTr[:, 3:6, :])
            nc.scalar.dma_start(wq_sb[:, 3:6, :], wqTr[:, 3:6, :])
            nc.gpsimd.dma_start(wk_sb[:, 3:6, :], wkTr[:, 3:6, :])
            nc.gpsimd.dma_start(wv_sb[:, 0:3, :], wvTr[:, 0:3, :])
            nc.gpsimd.dma_start(wv_sb[:, 3:6, :], wvTr[:, 3:6, :])
            nc.gpsimd.dma_start(wp_sb[:], wpTr[:])
            nc.scalar.dma_start(bpb_sb[:], bpb)

            # scratch for PE-warmup matmuls + ACT table preload
            scr = ph1.tile([P, 512], f16)
            scrt = ph1.tile([P, 16], f32)
            nc.vector.memset(scr[:], 0.01)
            nc.vector.memset(vp[:, :, :, D:2 * D], 1.0)
            # preload the exp table set (~2.7us) while input DMAs stream
            nc.scalar.activation(scrt[:], scr[:, 0:16], Exp, scale=1.0)
            # ~4.5us of dummy matmuls: HAM reaches full clock before the
            # first real matmul, whose inputs only land ~5us in
            wps = psp.tile([P, 512], f32, tag="qk", name="wps")
            for i in range(20):
                mm(wps[:], lhsT=scr[:, 0:128], rhs=scr[:],
                   start=(i == 0), stop=(i == 19))

            def emit_qk_group(j, gi):
                # gi 0..3 = (q,nh0), (k,nh0), (q,nh1), (k,nh1)
                w_sb, dst = ((wq_sb, qt), (wk_sb, kt))[gi % 2]
                nh = gi // 2
                ps = psp.tile([P, 512], f32, tag="qk", name="qkps")
                for c in range(CT):
                    mm(ps[:], lhsT=w_sb[:, c, j * P:(j + 1) * P],
                       rhs=x_sb[:, c, nh * 512:(nh + 1) * 512],
                       start=(c == 0), stop=(c == CT - 1))
                # the cast gates all of pair j's scores — preempt other
                # vector work (norms, V copies) the moment the psum lands
                with tc.high_priority(offset=1_000_000):
                    nc.vector.tensor_copy(
                        dst[:, j, nh * 512:(nh + 1) * 512], ps[:])

            def emit_v_t(t):
                psa = psp.tile([P, 512], f32, tag="qk", name="psa")
                psb = psp.tile([P, 512], f32, tag="qk", name="psb")
                for c in range(CT):
                    lh = x_sb[:, c, t * P:(t + 1) * P]
                    mm(psa[:], lhsT=lh, rhs=wv_sb[:, c, 0:512],
                       start=(c == 0), stop=(c == CT - 1))
                    mm(psb[:, 0:256], lhsT=lh, rhs=wv_sb[:, c, 512:768],
                       start=(c == 0), stop=(c == CT - 1))
                nc.vector.tensor_copy(
                    vp[:, t, 0:8, 0:D],
                    psa.rearrange("p (h d) -> p h d", d=D))
                nc.vector.tensor_copy(
                    vp[:, t, 8:12, 0:D],
                    psb[:, 0:256].rearrange("p (h d) -> p h d", d=D))

            ets = {}

            def emit_scores_mt(j, mt):
                # One PSUM tile per nh-half holds BOTH heads' scores
                # ([P, h0|h64, 512]), so each exp depends on both row-group
                # matmuls — the scheduler must keep the dual-stream pair
                # together instead of splitting it to unblock ACT early.
                s = {}
                for nh in range(2):
                    s[nh] = psp.tile([P, 2, 512], f32, tag="s",
                                     name=f"s_{nh}")
                    ets[(j, mt, nh)] = etp.tile([P, 2, 512], f16, tag="et",
                                                name=f"et_{nh}")
                # High priority: the exp stream paces the kernel, so score
                # matmuls must preempt filler groups the moment their PSUM
                # bank frees up — otherwise ACT gaps ~1us/step accumulate.
                with tc.high_priority(offset=1_000_000):
                    for nh in range(2):
                        for hh in range(2):   # adjacent => row-group dual
                            r0 = hh * D
                            mm(s[nh][:, hh, :],
                               lhsT=kt[r0:r0 + D, j, mt * P:(mt + 1) * P],
                               rhs=qt[r0:r0 + D, j,
                                      nh * 512:(nh + 1) * 512],
                               start=True, stop=True)
                    for nh in range(2):
                        nc.scalar.activation(ets[(j, mt, nh)][:], s[nh][:],
                                             Exp, scale=float(SCALE))

            def emit_pv_group(j, hh, nh):
                h = 2 * j + hh
                r0 = hh * D
                pso = psp.tile([P, 512], f32, tag="o", name="pso")
                for mt in range(NT):
                    mm(pso[:], lhsT=vp[:, mt, h],
                       rhs=ets[(j, mt, nh)][:, hh, :],
                       start=(mt == 0), stop=(mt == NT - 1))
                sums = smp.tile([D, 512], f32, tag="sums")
                rec = smp.tile([D, 512], f32, tag="rec")
                nc.vector.tensor_copy(sums[:], pso[D:2 * D, :])
                nc.vector.reciprocal_approx_fast(rec[:], sums[:])
                nc.vector.tensor_mul(
                    ot[r0:r0 + D, j, nh * 512:(nh + 1) * 512],
                    pso[0:D, :], rec[:])

            # ---- prologue: QK for pair 0 (DMA-paced) ----
            for gi in range(4):
                emit_qk_group(0, gi)

            # ---- filler queue: one PE group per score step, in dependency
            # order.  QK(j+1) groups (q=12 MMs, k=12 MMs) always land within
            # pair j's steps; V tiles precede PV(0); PV(j) follows et(j). ----
            # Slot constraints (PE queue is in-order, so violating either
            # deadlocks or stalls the queue):
            #  - pv(jj) group slots must be >= 8*(jj+1): only after pair
            #    jj's own score steps are all emitted may PV(jj) wait on
            #    its exps, else scores behind it starve ACT (circular wait).
            #  - qk(j) group slots must be < 8*j (scores(j,0) reads qt/kt).
            slot_map = {
                0: ("qk", 1, 0), 1: ("qk", 1, 1),
                2: ("v", 0), 3: ("v", 1), 4: ("v", 2), 5: ("v", 3),
                6: ("v", 4), 7: ("v", 5), 8: ("v", 6), 9: ("v", 7),
                10: ("pv", 0, 0, 0), 11: ("pv", 0, 0, 1),
                12: ("pv", 0, 1, 0), 14: ("qk", 2, 0), 15: ("qk", 2, 1),
                16: ("pv", 0, 1, 1),
                18: ("pv", 1, 0, 0), 19: ("pv", 1, 0, 1),
                20: ("pv", 1, 1, 0), 21: ("pv", 1, 1, 1),
                22: ("qk", 3, 0), 23: ("qk", 3, 1),
                24: ("pv", 2, 0, 0), 26: ("pv", 2, 0, 1),
                27: ("pv", 2, 1, 0), 29: ("pv", 2, 1, 1),
                30: ("qk", 4, 0), 31: ("qk", 4, 1),
                32: ("pv", 3, 0, 0), 34: ("pv", 3, 0, 1),
                35: ("pv", 3, 1, 0), 37: ("pv", 3, 1, 1),
                38: ("qk", 5, 0), 39: ("qk", 5, 1),
                41: ("pv", 4, 0, 0), 43: ("pv", 4, 0, 1),
                45: ("pv", 4, 1, 0), 47: ("pv", 4, 1, 1),
            }
            fillers = [slot_map.get(s) for s in range(HP * NT)]
            for s, f in enumerate(fillers):
                if f and f[0] == "pv":
                    assert s >= 8 * (f[1] + 1), (s, f)
                if f and f[0] == "qk":
                    assert s < 8 * f[1], (s, f)

            def emit_filler(f):
                if f is None:
                    return
                if f[0] == "qk":
                    _, j, half = f
                    emit_qk_group(j, 2 * half)      # (q or k, nh0)
                    emit_qk_group(j, 2 * half + 1)  # (q or k, nh1)
                elif f[0] == "v":
                    emit_v_t(f[1])
                else:
                    _, j, hh, nh = f
                    emit_pv_group(j, hh, nh)

            step = 0
            for j in range(HP):
                for mt in range(NT):
                    emit_scores_mt(j, mt)
                    emit_filler(fillers[step])
                    step += 1

            # ---- tail: PV of the last pair, then output projection ----
            for hh in range(2):
                for nh in range(2):
                    emit_pv_group(HP - 1, hh, nh)

            yre = y.rearrange("(t p) f -> t p f", p=P)
            for t in range(NT):
                yps = psp.tile([P, C], f32, tag="s", name="yps")
                for c in range(CT):
                    lh = ot[:, c, t * P:(t + 1) * P]
                    mm(yps[:, 0:512], lhsT=lh, rhs=wp_sb[:, c, 0:512],
                       start=(c == 0), stop=(c == CT - 1))
                    mm(yps[:, 512:768], lhsT=lh, rhs=wp_sb[:, c, 512:768],
                       start=(c == 0), stop=(c == CT - 1))
                ys = outp.tile([P, C], f32, tag="ys")
                nc.vector.tensor_add(ys[:, 0:512], yps[:, 0:512],
                                     bpb_sb[:, 0:512])
                nc.vector.tensor_add(ys[:, 512:768], yps[:, 512:768],
                                     bpb_sb[:, 512:768])
                eng = nc.sync if t % 2 == 0 else nc.scalar
                eng.dma_start(yre[t], ys[:])

    nc.compile()
    return nc


def _get_nc():
    if "nc" not in _cache:
        _cache["nc"] = _build_nc()
    return _cache["nc"]


def _make_in_maps(x, Wq, Wk, Wv, Wp, bp):
    x = np.asarray(x, dtype=np.float32)
    wqT = np.ascontiguousarray(
        np.asarray(Wq, np.float32).reshape(H * D, C).T.astype(np.float16))
    wkT = np.ascontiguousarray(
        np.asarray(Wk, np.float32).reshape(H * D, C).T.astype(np.float16))
    wvT = np.ascontiguousarray(
        np.asarray(Wv, np.float32).reshape(H * D, C).T.astype(np.float16))
    wpT = np.ascontiguousarray(
        np.asarray(Wp, np.float32).T.astype(np.float16))
    bpb = np.ascontiguousarray(
        np.broadcast_to(np.asarray(bp, np.float32), (P, C)))
    in_maps = []
    for b in range(NCORES):
        in_maps.append({
            "xT": np.ascontiguousarray(x[b].T.astype(np.float16)),
            "wqT": wqT, "wkT": wkT, "wvT": wvT, "wpT": wpT, "bpb": bpb,
        })
    return in_maps


def run(x, Wq, Wk, Wv, Wp, bp, trace=False):
    from concourse.bass_utils import run_bass_kernel_spmd
    nc = _get_nc()
    in_maps = _make_in_maps(x, Wq, Wk, Wv, Wp, bp)
    res = run_bass_kernel_spmd(nc, in_maps, list(range(NCORES)), trace=trace)
    out = np.stack([res.results[b]["y"] for b in range(NCORES)])
    return out, res


def kernel(x, Wq, Wk, Wv, Wp, bp):
    out, _ = run(x, Wq, Wk, Wv, Wp, bp)
    return out


# revision 17
# speedup vs baseline: 1.0209x; 1.0209x over previous
"""Trainium2 Bass kernel for nn_Attentionv2 (B=8, N=1024, C=768, H=12, D=64).

Strategy: data-parallel over batch — one batch element per NeuronCore (8 cores).
Per core, multi-head attention is computed entirely in the "transposed"
orientation so no on-chip transposes are needed:

  QT[h*64+d, n] = sum_c WqT[c, h*64+d] * xT[c, n]     (head-pair tiles)
  KT likewise; V[n, h*64+d] = sum_c xT[c, n-tile] * WvT[c, :]
  ST[m, n]  = sum_d KT[d, m] * QT[d, n]               (scores transposed;
               the two heads of a pair sit on partitions 0-63 / 64-127 so
               their K=64 matmuls dual-stream on the two PE row groups)
  ET        = exp(ST * 1/8)                            (no max-subtraction:
                                                        scores are O(1) here)
  PV lhsT   = [V_h | ones(64 cols)]  =>  out rows 0-63 = OT_h (unnorm),
               rows 64-127 = softmax denominator replicated 64x
  OT_norm   = OT * recip(Z)
  y[n, o]   = sum_c OT_norm[c, n] * WpT[c, o] + bp[o]

Pipelining: the kernel is paced by the ACT engine (96 exp calls ~110us).
Every score step (one (j,mt) pair: 2 dual-stream MMs + 2 exps) is followed
by exactly one filler group on the PE — V-projection t-tiles during pair 0,
then PV(j-1) quarter-groups and QK(j+1) quarter-groups — so neither PE nor
ACT ever starves and HAM stays at full clock.  Input DMAs are split into
prioritized chunks (x halves + w chunk-groups on 4 queues) so the first
exp issues ~11us in instead of ~56us.  One PSUM pool with 3 tags
(s:4 banks, qk:2, o:2) covers all phases — the output projection reuses
the score banks, avoiding a pool-boundary drain.

Matmul operands are fp16 (full-rate PE); accumulation is fp32 in PSUM.
"""

import numpy as np

P = 128
B, N, C = 8, 1024, 768
H, D = 12, 64
SCALE = D ** -0.5  # 0.125
CT = C // P   # 6 contraction chunks
NT = N // P   # 8 sequence tiles
HP = H // 2   # 6 head pairs
NCORES = 8

_cache = {}


def _build_nc():
    import concourse.bass as bass
    import concourse.mybir as mybir
    import concourse.tile as tile
    from concourse import bacc

    f32 = mybir.dt.float32
    f16 = mybir.dt.float16
    Exp = mybir.ActivationFunctionType.Exp

    nc = bacc.Bacc("TRN2", target_bir_lowering=False, debug=False,
                   enable_asserts=False)

    xT = nc.dram_tensor("xT", [C, N], f16, kind="ExternalInput").ap()
    wqT = nc.dram_tensor("wqT", [C, H * D], f16, kind="ExternalInput").ap()
    wkT = nc.dram_tensor("wkT", [C, H * D], f16, kind="ExternalInput").ap()
    wvT = nc.dram_tensor("wvT", [C, H * D], f16, kind="ExternalInput").ap()
    wpT = nc.dram_tensor("wpT", [C, C], f16, kind="ExternalInput").ap()
    bpb = nc.dram_tensor("bpb", [P, C], f32, kind="ExternalInput").ap()
    y = nc.dram_tensor("y", [N, C], f32, kind="ExternalOutput").ap()

    mm = nc.tensor.matmul

    xTr = xT.rearrange("(o p) n -> p o n", p=P)
    wqTr = wqT.rearrange("(o p) f -> p o f", p=P)
    wkTr = wkT.rearrange("(o p) f -> p o f", p=P)
    wvTr = wvT.rearrange("(o p) f -> p o f", p=P)
    wpTr = wpT.rearrange("(o p) f -> p o f", p=P)

    with tile.TileContext(nc) as tc:
        with tc.tile_pool(name="persist", bufs=1) as persist, \
             tc.tile_pool(name="ph1", bufs=1) as ph1, \
             tc.tile_pool(name="ps", bufs=1, space="PSUM") as psp, \
             tc.tile_pool(name="et", bufs=24) as etp, \
             tc.tile_pool(name="sm", bufs=4) as smp, \
             tc.tile_pool(name="outp", bufs=3) as outp:
            qt = persist.tile([P, HP, N], f16)
            kt = persist.tile([P, HP, N], f16)
            vp = persist.tile([P, NT, H, 2 * D], f16)  # [Vh | ones]
            ot = persist.tile([P, HP, N], f16)
            wp_sb = persist.tile([P, CT, C], f16)
            bpb_sb = persist.tile([P, C], f32)

            x_sb = ph1.tile([P, CT, N], f16)
            wq_sb = ph1.tile([P, CT, H * D], f16)
            wk_sb = ph1.tile([P, CT, H * D], f16)
            wv_sb = ph1.tile([P, CT, H * D], f16)

            # --- prioritized chunked input DMAs on 4 queues.  x halves +
            # wq/wk chunk-groups first (QK(0) gates the first exp), wv next
            # (needed by the V-proj fillers from ~13us), wp/bpb last. ---
            nc.sync.dma_start(x_sb[:, 0:3, :], xTr[:, 0:3, :])
            nc.scalar.dma_start(wq_sb[:, 0:3, :], wqTr[:, 0:3, :])
            nc.gpsimd.dma_start(wk_sb[:, 0:3, :], wkTr[:, 0:3, :])
            nc.sync.dma_start(x_sb[:, 3:6, :], xTr[:, 3:6, :])
            nc.scalar.dma_start(wq_sb[:, 3:6, :], wqTr[:, 3:6, :])
            nc.gpsimd.dma_start(wk_sb[:, 3:6, :], wkTr[:, 3:6, :])
            nc.gpsimd.dma_start(wv_sb[:, 0:3, :], wvTr[:, 0:3, :])
            nc.gpsimd.dma_start(wv_sb[:, 3:6, :], wvTr[:, 3:6, :])
            nc.gpsimd.dma_start(wp_sb[:], wpTr[:])
            nc.scalar.dma_start(bpb_sb[:], bpb)

            def at(us):
                # manual sim-time floor: dictates the Tile scheduler's
                # static per-engine order (runtime still uses semaphores)
                return tc.tile_wait_until(us / 1000.0)

            # scratch for PE-warmup matmuls + ACT table preload
            scr = ph1.tile([P, 512], f16)
            scrt = ph1.tile([P, 16], f32)
            nc.vector.memset(scr[:], 0.01)
            nc.vector.memset(vp[:, :, :, D:2 * D], 1.0)
            # preload the exp table set (~2.7us) while input DMAs stream
            nc.scalar.activation(scrt[:], scr[:, 0:16], Exp, scale=1.0)
            # Dummy matmuls keep HAM at full clock until real inputs land
            # (~20us): a free-running burst, then bursts gated on the wq/wk
            # chunk arrivals.  All write one scratch PSUM tile (tag "o";
            # PV only starts using that tag's rotation much later).
            wps = psp.tile([P, 512], f32, tag="o", name="wps")
            for i in range(20):
                mm(wps[:], lhsT=scr[:, 0:128], rhs=scr[:],
                   start=True, stop=True)
            with at(11.0):
                for i in range(12):
                    mm(wps[:], lhsT=scr[:, 0:128], rhs=wq_sb[:, 0, 0:512],
                       start=True, stop=True)
            with at(14.5):
                for i in range(8):
                    mm(wps[:], lhsT=scr[:, 0:128], rhs=wk_sb[:, 1, 0:512],
                       start=True, stop=True)

            def emit_qk_group(j, gi):
                # gi 0..3 = (q,nh0), (k,nh0), (q,nh1), (k,nh1)
                w_sb, dst = ((wq_sb, qt), (wk_sb, kt))[gi % 2]
                nh = gi // 2
                ps = psp.tile([P, 512], f32, tag="qk", name="qkps")
                for c in range(CT):
                    mm(ps[:], lhsT=w_sb[:, c, j * P:(j + 1) * P],
                       rhs=x_sb[:, c, nh * 512:(nh + 1) * 512],
                       start=(c == 0), stop=(c == CT - 1))
                # the cast gates all of pair j's scores — preempt other
                # vector work (norms, V copies) the moment the psum lands
                with tc.high_priority(offset=1_000_000):
                    nc.vector.tensor_copy(
                        dst[:, j, nh * 512:(nh + 1) * 512], ps[:])

            def emit_v_t(t):
                psa = psp.tile([P, 512], f32, tag="qk", name="psa")
                psb = psp.tile([P, 512], f32, tag="qk", name="psb")
                for c in range(CT):
                    lh = x_sb[:, c, t * P:(t + 1) * P]
                    mm(psa[:], lhsT=lh, rhs=wv_sb[:, c, 0:512],
                       start=(c == 0), stop=(c == CT - 1))
                    mm(psb[:, 0:256], lhsT=lh, rhs=wv_sb[:, c, 512:768],
                       start=(c == 0), stop=(c == CT - 1))
                nc.vector.tensor_copy(
                    vp[:, t, 0:8, 0:D],
                    psa.rearrange("p (h d) -> p h d", d=D))
                nc.vector.tensor_copy(
                    vp[:, t, 8:12, 0:D],
                    psb[:, 0:256].rearrange("p (h d) -> p h d", d=D))

            ets = {}

            def emit_scores_mt(j, mt):
                # One PSUM tile per nh-half holds BOTH heads' scores
                # ([P, h0|h64, 512]), so each exp depends on both row-group
                # matmuls — the scheduler must keep the dual-stream pair
                # together instead of splitting it to unblock ACT early.
                s = {}
                for nh in range(2):
                    s[nh] = psp.tile([P, 2, 512], f32, tag="s",
                                     name=f"s_{nh}")
                    ets[(j, mt, nh)] = etp.tile([P, 2, 512], f16, tag="et",
                                                name=f"et_{nh}")
                # High priority: the exp stream paces the kernel, so score
                # matmuls must preempt filler groups the moment their PSUM
                # bank frees up — otherwise ACT gaps ~1us/step accumulate.
                with tc.high_priority(offset=1_000_000):
                    for nh in range(2):
                        for hh in range(2):   # adjacent => row-group dual
                            r0 = hh * D
                            mm(s[nh][:, hh, :],
                               lhsT=kt[r0:r0 + D, j, mt * P:(mt + 1) * P],
                               rhs=qt[r0:r0 + D, j,
                                      nh * 512:(nh + 1) * 512],
                               start=True, stop=True)
                    for nh in range(2):
                        nc.scalar.activation(ets[(j, mt, nh)][:], s[nh][:],
                                             Exp, scale=float(SCALE))

            def emit_pv_group(j, hh, nh):
                h = 2 * j + hh
                r0 = hh * D
                pso = psp.tile([P, 512], f32, tag="o", name="pso")
                for mt in range(NT):
                    mm(pso[:], lhsT=vp[:, mt, h],
                       rhs=ets[(j, mt, nh)][:, hh, :],
                       start=(mt == 0), stop=(mt == NT - 1))
                sums = smp.tile([D, 512], f32, tag="sums")
                rec = smp.tile([D, 512], f32, tag="rec")
                nc.vector.tensor_copy(sums[:], pso[D:2 * D, :])
                nc.vector.reciprocal_approx_fast(rec[:], sums[:])
                nc.vector.tensor_mul(
                    ot[r0:r0 + D, j, nh * 512:(nh + 1) * 512],
                    pso[0:D, :], rec[:])

            # ---- prologue: QK for pair 0 (DMA-paced) ----
            for gi in range(4):
                with at(12.0 + 2.2 * gi):
                    emit_qk_group(0, gi)

            # ---- filler queue: one PE group per score step, in dependency
            # order.  QK(j+1) groups (q=12 MMs, k=12 MMs) always land within
            # pair j's steps; V tiles precede PV(0); PV(j) follows et(j). ----
            # Slot constraints (PE queue is in-order, so violating either
            # deadlocks or stalls the queue):
            #  - pv(jj) group slots must be >= 8*(jj+1): only after pair
            #    jj's own score steps are all emitted may PV(jj) wait on
            #    its exps, else scores behind it starve ACT (circular wait).
            #  - qk(j) group slots must be < 8*j (scores(j,0) reads qt/kt).
            slot_map = {
                0: ("qk", 1, 0), 1: ("qk", 1, 1),
                2: ("v", 0), 3: ("v", 1), 4: ("v", 2), 5: ("v", 3),
                6: ("v", 4), 7: ("v", 5), 8: ("v", 6), 9: ("v", 7),
                10: ("pv", 0, 0, 0), 11: ("pv", 0, 0, 1),
                12: ("pv", 0, 1, 0), 14: ("qk", 2, 0), 15: ("qk", 2, 1),
                16: ("pv", 0, 1, 1),
                18: ("pv", 1, 0, 0), 19: ("pv", 1, 0, 1),
                20: ("pv", 1, 1, 0), 21: ("pv", 1, 1, 1),
                22: ("qk", 3, 0), 23: ("qk", 3, 1),
                24: ("pv", 2, 0, 0), 26: ("pv", 2, 0, 1),
                27: ("pv", 2, 1, 0), 29: ("pv", 2, 1, 1),
                30: ("qk", 4, 0), 31: ("qk", 4, 1),
                32: ("pv", 3, 0, 0), 34: ("pv", 3, 0, 1),
                35: ("pv", 3, 1, 0), 37: ("pv", 3, 1, 1),
                38: ("qk", 5, 0), 39: ("qk", 5, 1),
                41: ("pv", 4, 0, 0), 43: ("pv", 4, 0, 1),
                45: ("pv", 4, 1, 0), 47: ("pv", 4, 1, 1),
            }
            fillers = [slot_map.get(s) for s in range(HP * NT)]
            for s, f in enumerate(fillers):
                if f and f[0] == "pv":
                    assert s >= 8 * (f[1] + 1), (s, f)
                if f and f[0] == "qk":
                    assert s < 8 * f[1], (s, f)

            def emit_filler(f):
                if f is None:
                    return
                if f[0] == "qk":
                    _, j, half = f
                    emit_qk_group(j, 2 * half)      # (q or k, nh0)
                    emit_qk_group(j, 2 * half + 1)  # (q or k, nh1)
                elif f[0] == "v":
                    emit_v_t(f[1])
                else:
                    _, j, hh, nh = f
                    emit_pv_group(j, hh, nh)

            # Timeline: score step s at T0+2.3s (ACT-paced), its filler
            # 0.7us later.  The manual sim-times pin the static per-engine
            # order to this fine-grained alternation — the scheduler's own
            # cost model misjudges dual-stream scores 2x and drifts.
            T0, STEP = 22.0, 2.3
            step = 0
            for j in range(HP):
                for mt in range(NT):
                    with at(T0 + STEP * step):
                        emit_scores_mt(j, mt)
                    with at(T0 + STEP * step + 0.7):
                        emit_filler(fillers[step])
                    step += 1

            # ---- tail: PV of the last pair, then output projection ----
            TEND = T0 + STEP * (HP * NT)
            for g, (hh, nh) in enumerate(((0, 0), (1, 0), (0, 1), (1, 1))):
                with at(TEND + 1.8 * g):
                    emit_pv_group(HP - 1, hh, nh)

            yre = y.rearrange("(t p) f -> t p f", p=P)
            for t in range(NT):
                with at(TEND + 7.2 + 1.95 * t):
                    yps = psp.tile([P, C], f32, tag="s", name="yps")
                    for c in range(CT):
                        lh = ot[:, c, t * P:(t + 1) * P]
                        mm(yps[:, 0:512], lhsT=lh, rhs=wp_sb[:, c, 0:512],
                           start=(c == 0), stop=(c == CT - 1))
                        mm(yps[:, 512:768], lhsT=lh,
                           rhs=wp_sb[:, c, 512:768],
                           start=(c == 0), stop=(c == CT - 1))
                    ys = outp.tile([P, C], f32, tag="ys")
                    nc.vector.tensor_add(ys[:, 0:512], yps[:, 0:512],
                                         bpb_sb[:, 0:512])
                    nc.vector.tensor_add(ys[:, 512:768], yps[:, 512:768],
                                         bpb_sb[:, 512:768])
                    eng = nc.sync if t % 2 == 0 else nc.scalar
                    eng.dma_start(yre[t], ys[:])

    nc.compile()
    return nc


def _get_nc():
    if "nc" not in _cache:
        _cache["nc"] = _build_nc()
    return _cache["nc"]


def _make_in_maps(x, Wq, Wk, Wv, Wp, bp):
    x = np.asarray(x, dtype=np.float32)
    wqT = np.ascontiguousarray(
        np.asarray(Wq, np.float32).reshape(H * D, C).T.astype(np.float16))
    wkT = np.ascontiguousarray(
        np.asarray(Wk, np.float32).reshape(H * D, C).T.astype(np.float16))
    wvT = np.ascontiguousarray(
        np.asarray(Wv, np.float32).reshape(H * D, C).T.astype(np.float16))
    wpT = np.ascontiguousarray(
        np.asarray(Wp, np.float32).T.astype(np.float16))
    bpb = np.ascontiguousarray(
        np.broadcast_to(np.asarray(bp, np.float32), (P, C)))
    in_maps = []
    for b in range(NCORES):
        in_maps.append({
            "xT": np.ascontiguousarray(x[b].T.astype(np.float16)),
            "wqT": wqT, "wkT": wkT, "wvT": wvT, "wpT": wpT, "bpb": bpb,
        })
    return in_maps


def run(x, Wq, Wk, Wv, Wp, bp, trace=False):
    from concourse.bass_utils import run_bass_kernel_spmd
    nc = _get_nc()
    in_maps = _make_in_maps(x, Wq, Wk, Wv, Wp, bp)
    res = run_bass_kernel_spmd(nc, in_maps, list(range(NCORES)), trace=trace)
    out = np.stack([res.results[b]["y"] for b in range(NCORES)])
    return out, res


def kernel(x, Wq, Wk, Wv, Wp, bp):
    out, _ = run(x, Wq, Wk, Wv, Wp, bp)
    return out


# revision 21
# speedup vs baseline: 1.0808x; 1.0586x over previous
"""Trainium2 Bass kernel for nn_Attentionv2 (B=8, N=1024, C=768, H=12, D=64).

Strategy: data-parallel over batch — one batch element per NeuronCore (8 cores).
Per core, multi-head attention is computed entirely in the "transposed"
orientation so no on-chip transposes are needed:

  QT[h*64+d, n] = sum_c WqT[c, h*64+d] * xT[c, n]     (head-pair tiles)
  KT likewise; V[n, h*64+d] = sum_c xT[c, n-tile] * WvT[c, :]
  ST[m, n]  = sum_d KT[d, m] * QT[d, n]               (scores transposed;
               the two heads of a pair sit on partitions 0-63 / 64-127 so
               their K=64 matmuls dual-stream on the two PE row groups)
  ET        = exp(ST * 1/8)                            (no max-subtraction:
                                                        scores are O(1) here)
  PV lhsT   = [V_h | ones(64 cols)]  =>  out rows 0-63 = OT_h (unnorm),
               rows 64-127 = softmax denominator replicated 64x
  OT_norm   = OT * recip(Z)
  y[n, o]   = sum_c OT_norm[c, n] * WpT[c, o] + bp[o]

Pipelining: the kernel is paced by the ACT engine (96 exp calls ~110us).
Every score step (one (j,mt) pair: 2 dual-stream MMs + 2 exps) is followed
by exactly one filler group on the PE — V-projection t-tiles during pair 0,
then PV(j-1) quarter-groups and QK(j+1) quarter-groups — so neither PE nor
ACT ever starves and HAM stays at full clock.  Input DMAs are split into
prioritized chunks (x halves + w chunk-groups on 4 queues) so the first
exp issues ~11us in instead of ~56us.  One PSUM pool with 3 tags
(s:4 banks, qk:2, o:2) covers all phases — the output projection reuses
the score banks, avoiding a pool-boundary drain.

Matmul operands are fp16 (full-rate PE); accumulation is fp32 in PSUM.
"""

import numpy as np

P = 128
B, N, C = 8, 1024, 768
H, D = 12, 64
SCALE = D ** -0.5  # 0.125
CT = C // P   # 6 contraction chunks
NT = N // P   # 8 sequence tiles
HP = H // 2   # 6 head pairs
NCORES = 8

_cache = {}


def _build_nc():
    import concourse.bass as bass
    import concourse.mybir as mybir
    import concourse.tile as tile
    from concourse import bacc

    f32 = mybir.dt.float32
    f16 = mybir.dt.float16
    Exp = mybir.ActivationFunctionType.Exp

    nc = bacc.Bacc("TRN2", target_bir_lowering=False, debug=False,
                   enable_asserts=False)

    xT = nc.dram_tensor("xT", [C, N], f16, kind="ExternalInput").ap()
    wqT = nc.dram_tensor("wqT", [C, H * D], f16, kind="ExternalInput").ap()
    wkT = nc.dram_tensor("wkT", [C, H * D], f16, kind="ExternalInput").ap()
    wvT = nc.dram_tensor("wvT", [C, H * D], f16, kind="ExternalInput").ap()
    wpT = nc.dram_tensor("wpT", [C, C], f16, kind="ExternalInput").ap()
    bpb = nc.dram_tensor("bpb", [P, C], f32, kind="ExternalInput").ap()
    y = nc.dram_tensor("y", [N, C], f32, kind="ExternalOutput").ap()

    mm = nc.tensor.matmul

    xTr = xT.rearrange("(o p) n -> p o n", p=P)
    wqTr = wqT.rearrange("(o p) f -> p o f", p=P)
    wkTr = wkT.rearrange("(o p) f -> p o f", p=P)
    wvTr = wvT.rearrange("(o p) f -> p o f", p=P)
    wpTr = wpT.rearrange("(o p) f -> p o f", p=P)

    with tile.TileContext(nc) as tc:
        with tc.tile_pool(name="persist", bufs=1) as persist, \
             tc.tile_pool(name="ph1", bufs=1) as ph1, \
             tc.tile_pool(name="ps", bufs=1, space="PSUM") as psp, \
             tc.tile_pool(name="et", bufs=40) as etp, \
             tc.tile_pool(name="sm", bufs=2) as smp, \
             tc.tile_pool(name="outp", bufs=2) as outp:
            qt = persist.tile([P, HP, N], f16)
            kt = persist.tile([P, HP, N], f16)
            vp = persist.tile([P, NT, H, 2 * D], f16)  # [Vh | ones]
            ot = persist.tile([P, HP, N], f16)
            wp_sb = persist.tile([P, CT, C], f16)
            bpb_sb = persist.tile([P, C], f32)

            x_sb = ph1.tile([P, CT, N], f16)
            wq_sb = ph1.tile([P, CT, H * D], f16)
            wk_sb = ph1.tile([P, CT, H * D], f16)
            wv_sb = ph1.tile([P, CT, H * D], f16)

            # --- prioritized chunked input DMAs on 4 queues.  x halves +
            # wq/wk chunk-groups first (QK(0) gates the first exp), wv next
            # (needed by the V-proj fillers from ~13us), wp/bpb last. ---
            nc.sync.dma_start(x_sb[:, 0:3, :], xTr[:, 0:3, :])
            nc.scalar.dma_start(wq_sb[:, 0:3, :], wqTr[:, 0:3, :])
            nc.gpsimd.dma_start(wk_sb[:, 0:3, :], wkTr[:, 0:3, :])
            nc.sync.dma_start(x_sb[:, 3:6, :], xTr[:, 3:6, :])
            nc.scalar.dma_start(wq_sb[:, 3:6, :], wqTr[:, 3:6, :])
            nc.gpsimd.dma_start(wk_sb[:, 3:6, :], wkTr[:, 3:6, :])
            nc.gpsimd.dma_start(wv_sb[:, 0:3, :], wvTr[:, 0:3, :])
            nc.gpsimd.dma_start(wv_sb[:, 3:6, :], wvTr[:, 3:6, :])
            nc.gpsimd.dma_start(wp_sb[:], wpTr[:])
            nc.scalar.dma_start(bpb_sb[:], bpb)

            def at(us):
                # manual sim-time floor: dictates the Tile scheduler's
                # static per-engine order (runtime still uses semaphores)
                return tc.tile_wait_until(us / 1000.0)

            # scratch for PE-warmup matmuls + ACT table preload
            scr = ph1.tile([P, 512], f16)
            scrt = ph1.tile([P, 16], f32)
            nc.vector.memset(scr[:], 0.01)
            nc.vector.memset(vp[:, :, :, D:2 * D], 1.0)
            # preload the exp table set (~2.7us) while input DMAs stream
            nc.scalar.activation(scrt[:], scr[:, 0:16], Exp, scale=1.0)
            # Dummy matmuls keep HAM at full clock until real inputs land
            # (~20us): a free-running burst, then bursts gated on the wq/wk
            # chunk arrivals.  All write one scratch PSUM tile (tag "o";
            # PV only starts using that tag's rotation much later).
            wps = psp.tile([P, 512], f32, tag="o", name="wps")
            for i in range(20):
                mm(wps[:], lhsT=scr[:, 0:128], rhs=scr[:],
                   start=True, stop=True)
            with at(11.0):
                for i in range(12):
                    mm(wps[:], lhsT=scr[:, 0:128], rhs=wq_sb[:, 0, 0:512],
                       start=True, stop=True)
            with at(14.5):
                for i in range(8):
                    mm(wps[:], lhsT=scr[:, 0:128], rhs=wk_sb[:, 1, 0:512],
                       start=True, stop=True)
            with at(17.5):
                for i in range(8):
                    mm(wps[:], lhsT=scr[:, 0:128], rhs=x_sb[:, 5, 0:512],
                       start=True, stop=True)

            def emit_qk_group(j, gi):
                # gi 0..3 = (q,nh0), (k,nh0), (q,nh1), (k,nh1)
                w_sb, dst = ((wq_sb, qt), (wk_sb, kt))[gi % 2]
                nh = gi // 2
                ps = psp.tile([P, 512], f32, tag="qk", name="qkps")
                for c in range(CT):
                    mm(ps[:], lhsT=w_sb[:, c, j * P:(j + 1) * P],
                       rhs=x_sb[:, c, nh * 512:(nh + 1) * 512],
                       start=(c == 0), stop=(c == CT - 1))
                # the cast gates all of pair j's scores — preempt other
                # vector work (norms, V copies) the moment the psum lands
                with tc.high_priority(offset=1_000_000):
                    nc.vector.tensor_copy(
                        dst[:, j, nh * 512:(nh + 1) * 512], ps[:])

            def emit_v_t(t):
                psa = psp.tile([P, 512], f32, tag="qk", name="psa")
                psb = psp.tile([P, 512], f32, tag="qk", name="psb")
                for c in range(CT):
                    lh = x_sb[:, c, t * P:(t + 1) * P]
                    mm(psa[:], lhsT=lh, rhs=wv_sb[:, c, 0:512],
                       start=(c == 0), stop=(c == CT - 1))
                    mm(psb[:, 0:256], lhsT=lh, rhs=wv_sb[:, c, 512:768],
                       start=(c == 0), stop=(c == CT - 1))
                nc.vector.tensor_copy(
                    vp[:, t, 0:8, 0:D],
                    psa.rearrange("p (h d) -> p h d", d=D))
                nc.vector.tensor_copy(
                    vp[:, t, 8:12, 0:D],
                    psb[:, 0:256].rearrange("p (h d) -> p h d", d=D))

            ets = {}

            def emit_scores_mt(j, mt):
                # One PSUM tile per nh-half holds BOTH heads' scores
                # ([P, h0|h64, 512]), so each exp depends on both row-group
                # matmuls — the scheduler must keep the dual-stream pair
                # together instead of splitting it to unblock ACT early.
                s = {}
                for nh in range(2):
                    s[nh] = psp.tile([P, 2, 512], f32, tag="s",
                                     name=f"s_{nh}")
                    ets[(j, mt, nh)] = etp.tile([P, 2, 512], f16, tag="et",
                                                name=f"et_{nh}")
                # High priority: the exp stream paces the kernel, so score
                # matmuls must preempt filler groups the moment their PSUM
                # bank frees up — otherwise ACT gaps ~1us/step accumulate.
                with tc.high_priority(offset=1_000_000):
                    for nh in range(2):
                        for hh in range(2):   # adjacent => row-group dual
                            r0 = hh * D
                            mm(s[nh][:, hh, :],
                               lhsT=kt[r0:r0 + D, j, mt * P:(mt + 1) * P],
                               rhs=qt[r0:r0 + D, j,
                                      nh * 512:(nh + 1) * 512],
                               start=True, stop=True)
                    for nh in range(2):
                        nc.scalar.activation(ets[(j, mt, nh)][:], s[nh][:],
                                             Exp, scale=float(SCALE))

            def emit_pv_group(j, hh, nh):
                h = 2 * j + hh
                r0 = hh * D
                pso = psp.tile([P, 512], f32, tag="o", name="pso")
                for mt in range(NT):
                    mm(pso[:], lhsT=vp[:, mt, h],
                       rhs=ets[(j, mt, nh)][:, hh, :],
                       start=(mt == 0), stop=(mt == NT - 1))
                sums = smp.tile([D, 512], f32, tag="sums")
                rec = smp.tile([D, 512], f32, tag="rec")
                nc.vector.tensor_copy(sums[:], pso[D:2 * D, :])
                nc.vector.reciprocal_approx_fast(rec[:], sums[:])
                nc.vector.tensor_mul(
                    ot[r0:r0 + D, j, nh * 512:(nh + 1) * 512],
                    pso[0:D, :], rec[:])

            # ---- prologue: QK for pair 0 (DMA-paced) ----
            for gi in range(4):
                with at(12.0 + 2.2 * gi):
                    emit_qk_group(0, gi)

            # ---- filler queue: one PE group per score step, in dependency
            # order.  QK(j+1) groups (q=12 MMs, k=12 MMs) always land within
            # pair j's steps; V tiles precede PV(0); PV(j) follows et(j). ----
            # Slot constraints (PE queue is in-order, so violating either
            # deadlocks or stalls the queue):
            #  - pv(jj) group slots must be >= 8*(jj+1): only after pair
            #    jj's own score steps are all emitted may PV(jj) wait on
            #    its exps, else scores behind it starve ACT (circular wait).
            #  - qk(j) group slots must be < 8*j (scores(j,0) reads qt/kt).
            # One single filler group per score step, loads balanced per
            # pair (~17.8us PE vs 18.3us ACT): V split across j=0-1,
            # PV(jj) one pair later than minimal so every j carries ~14us
            # of filler.  et bufs=40 covers PV's 19-step lag.
            fillers = []
            fillers += [("qk", 1, g) for g in range(4)]        # j=0
            fillers += [("v", t) for t in range(4)]
            fillers += [("v", t) for t in range(4, 8)]         # j=1
            fillers += [("qk", 2, g) for g in range(4)]
            for jj in range(3):                                # j=2..4
                fillers += [("pv", jj, g // 2, g % 2) for g in range(4)]
                fillers += [("qk", jj + 3, g) for g in range(4)]
            fillers += [("pv", 3, g // 2, g % 2) for g in range(4)]  # j=5
            fillers += [("pv", 4, g // 2, g % 2) for g in range(4)]
            assert len(fillers) == HP * NT
            for s, f in enumerate(fillers):
                if f[0] == "pv":
                    assert s >= 8 * (f[1] + 1), (s, f)
                if f[0] == "qk":
                    assert s < 8 * f[1], (s, f)

            def emit_filler(f):
                if f[0] == "qk":
                    emit_qk_group(f[1], f[2])
                elif f[0] == "v":
                    emit_v_t(f[1])
                else:
                    _, j, hh, nh = f
                    emit_pv_group(j, hh, nh)

            # Timeline: score step s at T0+2.3s (ACT-paced), its filler
            # 0.7us later.  The manual sim-times pin the static per-engine
            # order to this fine-grained alternation — the scheduler's own
            # cost model misjudges dual-stream scores 2x and drifts.
            T0, STEP = 22.0, 2.3
            step = 0
            for j in range(HP):
                for mt in range(NT):
                    with at(T0 + STEP * step):
                        emit_scores_mt(j, mt)
                    with at(T0 + STEP * step + 0.7):
                        emit_filler(fillers[step])
                    step += 1

            # ---- tail: PV of the last pair, then output projection ----
            TEND = T0 + STEP * (HP * NT)
            for g, (hh, nh) in enumerate(((0, 0), (1, 0), (0, 1), (1, 1))):
                with at(TEND + 1.8 * g):
                    emit_pv_group(HP - 1, hh, nh)

            yre = y.rearrange("(t p) f -> t p f", p=P)
            for t in range(NT):
                with at(TEND + 7.2 + 1.95 * t):
                    yps = psp.tile([P, C], f32, tag="s", name="yps")
                    for c in range(CT):
                        lh = ot[:, c, t * P:(t + 1) * P]
                        mm(yps[:, 0:512], lhsT=lh, rhs=wp_sb[:, c, 0:512],
                           start=(c == 0), stop=(c == CT - 1))
                        mm(yps[:, 512:768], lhsT=lh,
                           rhs=wp_sb[:, c, 512:768],
                           start=(c == 0), stop=(c == CT - 1))
                    ys = outp.tile([P, C], f32, tag="ys")
                    nc.vector.tensor_add(ys[:, 0:512], yps[:, 0:512],
                                         bpb_sb[:, 0:512])
                    nc.vector.tensor_add(ys[:, 512:768], yps[:, 512:768],
                                         bpb_sb[:, 512:768])
                    eng = nc.sync if t % 2 == 0 else nc.scalar
                    eng.dma_start(yre[t], ys[:])

    nc.compile()
    return nc


def _get_nc():
    if "nc" not in _cache:
        _cache["nc"] = _build_nc()
    return _cache["nc"]


def _make_in_maps(x, Wq, Wk, Wv, Wp, bp):
    x = np.asarray(x, dtype=np.float32)
    wqT = np.ascontiguousarray(
        np.asarray(Wq, np.float32).reshape(H * D, C).T.astype(np.float16))
    wkT = np.ascontiguousarray(
        np.asarray(Wk, np.float32).reshape(H * D, C).T.astype(np.float16))
    wvT = np.ascontiguousarray(
        np.asarray(Wv, np.float32).reshape(H * D, C).T.astype(np.float16))
    wpT = np.ascontiguousarray(
        np.asarray(Wp, np.float32).T.astype(np.float16))
    bpb = np.ascontiguousarray(
        np.broadcast_to(np.asarray(bp, np.float32), (P, C)))
    in_maps = []
    for b in range(NCORES):
        in_maps.append({
            "xT": np.ascontiguousarray(x[b].T.astype(np.float16)),
            "wqT": wqT, "wkT": wkT, "wvT": wvT, "wpT": wpT, "bpb": bpb,
        })
    return in_maps


def run(x, Wq, Wk, Wv, Wp, bp, trace=False):
    from concourse.bass_utils import run_bass_kernel_spmd
    nc = _get_nc()
    in_maps = _make_in_maps(x, Wq, Wk, Wv, Wp, bp)
    res = run_bass_kernel_spmd(nc, in_maps, list(range(NCORES)), trace=trace)
    out = np.stack([res.results[b]["y"] for b in range(NCORES)])
    return out, res


def kernel(x, Wq, Wk, Wv, Wp, bp):
    out, _ = run(x, Wq, Wk, Wv, Wp, bp)
    return out


# revision 27
# speedup vs baseline: 1.3027x; 1.2054x over previous
"""Trainium2 Bass kernel for nn_Attentionv2 (B=8, N=1024, C=768, H=12, D=64).

Strategy: data-parallel over batch — one batch element per NeuronCore (8 cores).
Per core, multi-head attention is computed entirely in the "transposed"
orientation so no on-chip transposes are needed:

  QT[h*64+d, n] = sum_c WqT[c, h*64+d] * xT[c, n]     (head-pair tiles)
  KT likewise; V[n, h*64+d] = sum_c xT[c, n-tile] * WvT[c, :]
  ST[m, n]  = sum_d KT[d, m] * QT[d, n]               (scores transposed;
               the two heads of a pair sit on partitions 0-63 / 64-127 so
               their K=64 matmuls row-tile into the two PE array halves)
  ET        = exp(ST * 1/8)                            (no max-subtraction:
                                                        scores are O(1) here)
  PV lhsT   = [V_h | ones(64 cols)]  =>  out rows 0-63 = OT_h (unnorm),
               rows 64-127 = softmax denominator replicated 64x (free
               partition-broadcast done by the PE)
  OT_norm   = OT * exp(-ln(Z))                         (reciprocal via ACT)
  y[n, o]   = sum_c OT_norm[c, n] * WpT[c, o] + bp[o]

Matmul operands are fp16 (full-rate PE, fast weight loads, HAM-warm clocks);
all accumulation is fp32 in PSUM.
"""

import numpy as np

P = 128
B, N, C = 8, 1024, 768
H, D = 12, 64
SCALE = D ** -0.5  # 0.125
CT = C // P   # 6 contraction chunks
NT = N // P   # 8 sequence tiles
HP = H // 2   # 6 head pairs
NCORES = 8

_cache = {}


def _build_nc():
    import concourse.bass as bass
    import concourse.mybir as mybir
    import concourse.tile as tile
    from concourse import bacc

    f32 = mybir.dt.float32
    f16 = mybir.dt.float16
    Exp = mybir.ActivationFunctionType.Exp
    Ln = mybir.ActivationFunctionType.Ln

    nc = bacc.Bacc("TRN2", target_bir_lowering=False, debug=False,
                   enable_asserts=False)

    xT = nc.dram_tensor("xT", [C, N], f16, kind="ExternalInput").ap()
    wqT = nc.dram_tensor("wqT", [C, H * D], f16, kind="ExternalInput").ap()
    wkT = nc.dram_tensor("wkT", [C, H * D], f16, kind="ExternalInput").ap()
    wvT = nc.dram_tensor("wvT", [C, H * D], f16, kind="ExternalInput").ap()
    wpT = nc.dram_tensor("wpT", [C, C], f16, kind="ExternalInput").ap()
    bpb = nc.dram_tensor("bpb", [P, C], f32, kind="ExternalInput").ap()
    y = nc.dram_tensor("y", [N, C], f32, kind="ExternalOutput").ap()

    mm = nc.tensor.matmul

    xTr = xT.rearrange("(o p) n -> p o n", p=P)
    wqTr = wqT.rearrange("(o p) f -> p o f", p=P)
    wkTr = wkT.rearrange("(o p) f -> p o f", p=P)
    wvTr = wvT.rearrange("(o p) f -> p o f", p=P)
    wpTr = wpT.rearrange("(o p) f -> p o f", p=P)

    with tile.TileContext(nc) as tc:
        with tc.tile_pool(name="persist", bufs=1) as persist:
            qt = persist.tile([P, HP, N], f16)        # QT: head pair j rows
            kt = persist.tile([P, HP, N], f16)
            vp = persist.tile([P, NT, H, 2 * D], f16)  # [Vh | ones]
            ot = persist.tile([P, HP, N], f16)        # normalized OT stacked
            wp_sb = persist.tile([P, CT, C], f16)
            bpb_sb = persist.tile([P, C], f32)

            nc.vector.memset(vp[:, :, :, D:2 * D], 1.0)

            # ---- Phases 1+2: projections + attention, interleaved.
            # V and QK(pair 0) run up front; QK(pair j+1) is emitted inside
            # pair j's attention block as dense PE filler that keeps HAM
            # warm during the ACT-paced exp stretches. ----
            with tc.tile_pool(name="ph1", bufs=1) as ph1, \
                 tc.tile_pool(name="mix", bufs=2, space="PSUM") as mix, \
                 tc.tile_pool(name="et", bufs=24) as etp, \
                 tc.tile_pool(name="sm", bufs=4) as smp, \
                 tc.tile_pool(name="ps_s", bufs=2, space="PSUM") as ps_s, \
                 tc.tile_pool(name="ps_o", bufs=2, space="PSUM") as ps_o:
                x_sb = ph1.tile([P, CT, N], f16)
                wq_sb = ph1.tile([P, CT, H * D], f16)
                wk_sb = ph1.tile([P, CT, H * D], f16)
                wv_sb = ph1.tile([P, CT, H * D], f16)
                # Chunked, priority-ordered input DMAs on 3 queues: the
                # first exp is gated on x+wq+wk (3.75MB, ~20us incl the
                # ~9us DMA-path startup); wv/wp/bpb queue up behind.
                nc.sync.dma_start(x_sb[:, 0:3, :], xTr[:, 0:3, :])
                nc.scalar.dma_start(wq_sb[:, 0:3, :], wqTr[:, 0:3, :])
                nc.gpsimd.dma_start(wk_sb[:, 0:3, :], wkTr[:, 0:3, :])
                nc.sync.dma_start(x_sb[:, 3:6, :], xTr[:, 3:6, :])
                nc.scalar.dma_start(wq_sb[:, 3:6, :], wqTr[:, 3:6, :])
                nc.gpsimd.dma_start(wk_sb[:, 3:6, :], wkTr[:, 3:6, :])
                nc.gpsimd.dma_start(wv_sb[:, 0:3, :], wvTr[:, 0:3, :])
                nc.gpsimd.dma_start(wv_sb[:, 3:6, :], wvTr[:, 3:6, :])
                nc.gpsimd.dma_start(wp_sb[:], wpTr[:])
                nc.scalar.dma_start(bpb_sb[:], bpb)

                def at(us):
                    # sim-time floor: places instructions in the static
                    # per-engine order without affecting runtime waits
                    return tc.tile_wait_until(us / 1000.0)

                # scratch + exp-table preload + HAM warmup matmuls.  The
                # dummy matmuls (one WAW scratch psum tile, tag "o" whose
                # rotation PV only reaches much later) keep the PE clock at
                # full rate across the input-DMA window.
                scr = ph1.tile([P, 512], f16)
                scrt = ph1.tile([P, 16], f32)
                nc.vector.memset(scr[:], 0.01)
                nc.scalar.activation(scrt[:], scr[:, 0:16], Exp, scale=1.0)
                wps = ps_o.tile([P, 512], f32, tag="o", name="wps")
                for i in range(20):
                    mm(wps[:], lhsT=scr[:, 0:128], rhs=scr[:],
                       start=True, stop=True)
                with at(11.0):
                    for i in range(12):
                        mm(wps[:], lhsT=scr[:, 0:128],
                           rhs=wq_sb[:, 0, 0:512], start=True, stop=True)
                with at(14.5):
                    for i in range(8):
                        mm(wps[:], lhsT=scr[:, 0:128],
                           rhs=wk_sb[:, 1, 0:512], start=True, stop=True)
                with at(17.5):
                    for i in range(8):
                        mm(wps[:], lhsT=scr[:, 0:128],
                           rhs=x_sb[:, 5, 0:512], start=True, stop=True)

                def emit_qk(j):
                    for w_sb, dst in ((wq_sb, qt), (wk_sb, kt)):
                        for nh in range(2):
                            ps = mix.tile([P, 512], f32, tag="qk",
                                          name="qkps")
                            for c in range(CT):
                                mm(ps[:], lhsT=w_sb[:, c, j * P:(j + 1) * P],
                                   rhs=x_sb[:, c, nh * 512:(nh + 1) * 512],
                                   start=(c == 0), stop=(c == CT - 1))
                            nc.vector.tensor_copy(
                                dst[:, j, nh * 512:(nh + 1) * 512], ps[:])

                emit_qk(0)
                for t in range(NT):
                    psa = mix.tile([P, 512], f32, tag="qk", name="psa")
                    psb = mix.tile([P, 512], f32, tag="qk", name="psb")
                    for c in range(CT):
                        lh = x_sb[:, c, t * P:(t + 1) * P]
                        mm(psa[:], lhsT=lh, rhs=wv_sb[:, c, 0:512],
                           start=(c == 0), stop=(c == CT - 1))
                        mm(psb[:, 0:256], lhsT=lh, rhs=wv_sb[:, c, 512:768],
                           start=(c == 0), stop=(c == CT - 1))
                    nc.vector.tensor_copy(
                        vp[:, t, 0:8, 0:D],
                        psa.rearrange("p (h d) -> p h d", d=D))
                    nc.vector.tensor_copy(
                        vp[:, t, 8:12, 0:D],
                        psb[:, 0:256].rearrange("p (h d) -> p h d", d=D))

                ets = {}

                def emit_scores_mt(j, mt):
                    # Both heads' scores for one nh-half share one PSUM
                    # tile, so each exp depends on both row-group matmuls
                    # and the scheduler cannot split the dual-stream pair.
                    s = {}
                    for nh in range(2):
                        s[nh] = ps_s.tile([P, 2, 512], f32, tag="s",
                                          name=f"s_{nh}")
                        ets[(j, mt, nh)] = etp.tile([P, 2, 512], f16,
                                                    tag="et", name=f"et_{nh}")
                    for nh in range(2):
                        for hh in range(2):   # adjacent => PE row-tiling
                            r0 = hh * D
                            mm(s[nh][:, hh, :],
                               lhsT=kt[r0:r0 + D, j, mt * P:(mt + 1) * P],
                               rhs=qt[r0:r0 + D, j, nh * 512:(nh + 1) * 512],
                               start=True, stop=True)
                    for nh in range(2):
                        nc.scalar.activation(ets[(j, mt, nh)][:], s[nh][:],
                                             Exp, scale=float(SCALE))

                def emit_pv_norm(j):
                    for hh in range(2):
                        h = 2 * j + hh
                        r0 = hh * D
                        pso = {nh: ps_o.tile([P, 512], f32, tag="o",
                                             name=f"o_{nh}")
                               for nh in range(2)}
                        for mt in range(NT):   # dense 16-MM PV burst
                            for nh in range(2):
                                mm(pso[nh][:],
                                   lhsT=vp[:, mt, h],
                                   rhs=ets[(j, mt, nh)][:, hh, :],
                                   start=(mt == 0), stop=(mt == NT - 1))
                        for nh in range(2):
                            sums = smp.tile([D, 512], f32, tag="sums")
                            rec = smp.tile([D, 512], f32, tag="rec")
                            nc.vector.tensor_copy(sums[:],
                                                  pso[nh][D:2 * D, :])
                            nc.vector.reciprocal_approx_fast(rec[:], sums[:])
                            nc.vector.tensor_mul(
                                ot[r0:r0 + D, j, nh * 512:(nh + 1) * 512],
                                pso[nh][0:D, :], rec[:])

                # software-pipelined: PV/normalize of pair j-1 lands after
                # pair j's first score steps so ACT never stalls at pair
                # boundaries; QK of pair j+1 fills mid-pair PE gaps.
                for j in range(HP):
                    for mt in range(NT):
                        emit_scores_mt(j, mt)
                        if mt == 1:
                            if j > 0:
                                emit_pv_norm(j - 1)
                            if j + 1 < HP:
                                emit_qk(j + 1)
                emit_pv_norm(HP - 1)

            # ---- Phase 3: output projection ----
            with tc.tile_pool(name="outp", bufs=3) as outp, \
                 tc.tile_pool(name="ps_y", bufs=4, space="PSUM") as ps_y:
                yre = y.rearrange("(t p) f -> t p f", p=P)
                for t in range(NT):
                    pa = ps_y.tile([P, 512], f32, tag="y")
                    pb = ps_y.tile([P, 512], f32, tag="y")
                    for c in range(CT):
                        lh = ot[:, c, t * P:(t + 1) * P]
                        mm(pa[:], lhsT=lh, rhs=wp_sb[:, c, 0:512],
                           start=(c == 0), stop=(c == CT - 1))
                        mm(pb[:, 0:256], lhsT=lh, rhs=wp_sb[:, c, 512:768],
                           start=(c == 0), stop=(c == CT - 1))
                    ys = outp.tile([P, C], f32, tag="ys")
                    nc.vector.tensor_add(ys[:, 0:512], pa[:], bpb_sb[:, 0:512])
                    nc.vector.tensor_add(ys[:, 512:768], pb[:, 0:256],
                                         bpb_sb[:, 512:768])
                    eng = nc.sync if t % 2 == 0 else nc.scalar
                    eng.dma_start(yre[t], ys[:])

    nc.compile()
    return nc


def _get_nc():
    if "nc" not in _cache:
        _cache["nc"] = _build_nc()
    return _cache["nc"]


def _make_in_maps(x, Wq, Wk, Wv, Wp, bp):
    x = np.asarray(x, dtype=np.float32)
    wqT = np.ascontiguousarray(
        np.asarray(Wq, np.float32).reshape(H * D, C).T.astype(np.float16))
    wkT = np.ascontiguousarray(
        np.asarray(Wk, np.float32).reshape(H * D, C).T.astype(np.float16))
    wvT = np.ascontiguousarray(
        np.asarray(Wv, np.float32).reshape(H * D, C).T.astype(np.float16))
    wpT = np.ascontiguousarray(
        np.asarray(Wp, np.float32).T.astype(np.float16))
    bpb = np.ascontiguousarray(
        np.broadcast_to(np.asarray(bp, np.float32), (P, C)))
    in_maps = []
    for b in range(NCORES):
        in_maps.append({
            "xT": np.ascontiguousarray(x[b].T.astype(np.float16)),
            "wqT": wqT, "wkT": wkT, "wvT": wvT, "wpT": wpT, "bpb": bpb,
        })
    return in_maps


def run(x, Wq, Wk, Wv, Wp, bp, trace=False):
    from concourse.bass_utils import run_bass_kernel_spmd
    nc = _get_nc()
    in_maps = _make_in_maps(x, Wq, Wk, Wv, Wp, bp)
    res = run_bass_kernel_spmd(nc, in_maps, list(range(NCORES)), trace=trace)
    out = np.stack([res.results[b]["y"] for b in range(NCORES)])
    return out, res


def kernel(x, Wq, Wk, Wv, Wp, bp):
    out, _ = run(x, Wq, Wk, Wv, Wp, bp)
    return out

